# revision 1
# baseline (speedup 1.0000x reference)
"""Trainium2 Bass kernel: 2-layer GRU (B=64, S=4096, H=128) + MLP head + cumsum.

Sharding: data-parallel over batch -> 8 cores x 8 sequences, weights replicated.

Per-core design (layout: hidden dim on the 128 SBUF partitions, batch on free dim):
  - One fused "superstep" tau computes GRU layer0 @ t=tau and layer1 @ t=tau-1
    (one-step skew so layer1's input h1_{tau-1} is already available).
  - All gate preactivations for both layers land in ONE psum tile [128, 64]
    (8 col-blocks of 8: r0 z0 ni0 nh0 r1 z1 ni1 nh1) via 13 small matmuls:
      bias outer-product MM (K=5 one-hot), 3 x-side MMs per layer, 3 h-side
      MMs per layer.  Layer-0 x-side uses a 17-row augmented input (ones row
      carries the r/z/ni biases).
  - sigmoid over [r0 z0 r1 z1] in one ACT op reading PSUM; n-gate:
    t = r * psum_nh;  s = t + psum_ni;  n = tanh(s);
    blend: d = h_prev - n; e = z * d; h' = n + e  (VectorE).
  - h' [ha_tau | hb_{tau-1}] written into a rolling per-chunk history buffer;
    the MLP head consumes hb + x in bulk per 512-step chunk (big matmuls),
    and the final cumsum is one tensor_tensor_scan.
"""

import os
import numpy as np

I_ = 16
H_ = 128
B_ = 64
S_FULL = 4096
MLP_ = 64
NCORES = 8
BL = 8  # batch per core

# stash of the last run's profiling info for test.py
LAST = {}
DEBUG_DUMP = False
DEBUG_TAU = 0


def _build_program(S, C, b2f, imf):
    import concourse.bacc as bacc
    import concourse.tile as tile
    from concourse import mybir

    F32 = mybir.dt.float32
    AF = mybir.ActivationFunctionType
    OP = mybir.AluOpType

    assert S % C == 0
    nchunks = S // C
    assert C % 64 == 0

    nc = bacc.Bacc("TRN2", target_bir_lowering=False, debug=False,
                   num_devices=NCORES)

    xt_d = nc.dram_tensor("xt", [17, BL * S], F32, kind="ExternalInput").ap()
    wih0_d = nc.dram_tensor("wih0", [17, 3 * H_], F32, kind="ExternalInput").ap()
    whh0_d = nc.dram_tensor("whh0", [H_, 3 * H_], F32, kind="ExternalInput").ap()
    wih1_d = nc.dram_tensor("wih1", [H_, 3 * H_], F32, kind="ExternalInput").ap()
    whh1_d = nc.dram_tensor("whh1", [H_, 3 * H_], F32, kind="ExternalInput").ap()
    bT_d = nc.dram_tensor("biasT", [5, H_], F32, kind="ExternalInput").ap()
    bsel_d = nc.dram_tensor("bsel", [5, 64], F32, kind="ExternalInput").ap()
    w1h_d = nc.dram_tensor("w1h", [H_, MLP_], F32, kind="ExternalInput").ap()
    w1x_d = nc.dram_tensor("w1x", [17, MLP_], F32, kind="ExternalInput").ap()
    w2_d = nc.dram_tensor("w2", [MLP_, 1], F32, kind="ExternalInput").ap()
    y_d = nc.dram_tensor("y", [BL, S], F32, kind="ExternalOutput").ap()
    dbg_hab_d = dbg_inc_d = None
    if DEBUG_DUMP:
        dbg_hab_d = nc.dram_tensor("dbg_hab", [128, C * 16], F32,
                                   kind="ExternalOutput").ap()
        dbg_inc_d = nc.dram_tensor("dbg_inc", [BL, S], F32,
                                   kind="ExternalOutput").ap()

    with tile.TileContext(nc) as tc, \
         tc.tile_pool(name="const", bufs=1) as constp, \
         tc.tile_pool(name="xtp", bufs=3) as xtp, \
         tc.tile_pool(name="habp", bufs=2) as habp, \
         tc.tile_pool(name="workp", bufs=3) as workp, \
         tc.tile_pool(name="hdnp", bufs=3) as hdnp, \
         tc.tile_pool(name="thp", bufs=3) as thp, \
         tc.tile_pool(name="bigp", bufs=1) as bigp, \
         tc.tile_pool(name="psp", bufs=4, space="PSUM") as psp, \
         tc.tile_pool(name="pshp", bufs=2, space="PSUM") as pshp, \
         tc.tile_pool(name="psip", bufs=2, space="PSUM") as psip:

        # ---------------- constants / weights ----------------
        def load_const(name, dram, shape):
            t = constp.tile(shape, F32, tag=name)
            nc.sync.dma_start(out=t[:, :], in_=dram)
            return t

        wih0 = load_const("wih0", wih0_d, [17, 3 * H_])
        whh0 = load_const("whh0", whh0_d, [H_, 3 * H_])
        wih1 = load_const("wih1", wih1_d, [H_, 3 * H_])
        whh1 = load_const("whh1", whh1_d, [H_, 3 * H_])
        bT = load_const("biasT", bT_d, [5, H_])
        bsel = load_const("bsel", bsel_d, [5, 64])
        w1h = load_const("w1h", w1h_d, [H_, MLP_])
        w1x = load_const("w1x", w1x_d, [17, MLP_])
        w2 = load_const("w2", w2_d, [MLP_, 1])

        zero16 = constp.tile([128, 16], F32, tag="zero16", name="zero16")
        nc.vector.memset(zero16[:, :], 0.0)
        b2t = constp.tile([1, 1], F32, tag="b2t", name="b2t")
        nc.vector.memset(b2t[:, :], b2f)
        prol = constp.tile([128, 16], F32, tag="prol", name="prol")
        nc.vector.memset(prol[:, :], 0.0)

        inc_sb = bigp.tile([BL, S], F32, tag="inc", name="inc")

        hab_tiles = {}
        xt_tiles = {}

        def xt_load(k):
            if k >= nchunks or k in xt_tiles:
                return
            t = xtp.tile([17, BL, C], F32, tag="xtc", name="xtc")
            src = xt_d.rearrange("k (b t) -> k b t", b=BL)[:, :, k * C:(k + 1) * C]
            nc.sync.dma_start(out=t[:, :, :], in_=src)
            xt_tiles[k] = t

        def entry_ap(tau):
            # hab entry written at superstep tau: [ha_tau | hb_{tau-1}]
            if tau == 0:
                return prol[:, :]
            k = (tau - 1) // C
            e = (tau - 1) % C
            return hab_tiles[k][:, e, :]

        GR, GZ, GN = 0, 1, 2

        def superstep(tau):
            ps = psp.tile([128, 64], F32, tag="ps", name="ps")
            hprev = zero16[:, :] if tau == 0 else entry_ap(tau - 1)
            ha = hprev[:, 0:8]   # h of layer0 at tau-1 (= layer1 input)
            hb = hprev[:, 8:16]  # h of layer1 at tau-2

            t0 = min(tau, S - 1)
            xrhs = xt_tiles[t0 // C][:, :, t0 % C]  # [17, 8]

            mm = nc.tensor.matmul
            # bias outer-product opens the ONLY psum accumulation group for
            # this bank: start=True zeroes the whole 2KB zero-region, so it
            # must cover all 64 cols (bsel has zero cols outside the bias
            # blocks) and every other matmul accumulates with start=False.
            mm(ps[:, 0:64], lhsT=bT[:, :], rhs=bsel[:, :],
               start=True, stop=False)
            acc = dict(start=False, stop=False, skip_group_check=True)
            # layer0 x-side (K=17, aug row carries r/z/ni biases)
            mm(ps[:, 0:8], lhsT=wih0[:, GR * H_:(GR + 1) * H_], rhs=xrhs, **acc)
            mm(ps[:, 8:16], lhsT=wih0[:, GZ * H_:(GZ + 1) * H_], rhs=xrhs, **acc)
            mm(ps[:, 16:24], lhsT=wih0[:, GN * H_:(GN + 1) * H_], rhs=xrhs, **acc)
            # layer0 h-side
            mm(ps[:, 0:8], lhsT=whh0[:, GR * H_:(GR + 1) * H_], rhs=ha, **acc)
            mm(ps[:, 8:16], lhsT=whh0[:, GZ * H_:(GZ + 1) * H_], rhs=ha, **acc)
            mm(ps[:, 24:32], lhsT=whh0[:, GN * H_:(GN + 1) * H_], rhs=ha, **acc)
            # layer1 x-side (input = ha)
            mm(ps[:, 32:40], lhsT=wih1[:, GR * H_:(GR + 1) * H_], rhs=ha, **acc)
            mm(ps[:, 40:48], lhsT=wih1[:, GZ * H_:(GZ + 1) * H_], rhs=ha, **acc)
            mm(ps[:, 48:56], lhsT=wih1[:, GN * H_:(GN + 1) * H_], rhs=ha, **acc)
            # layer1 h-side; the last matmul closes the accumulation group
            mm(ps[:, 32:40], lhsT=whh1[:, GR * H_:(GR + 1) * H_], rhs=hb, **acc)
            mm(ps[:, 40:48], lhsT=whh1[:, GZ * H_:(GZ + 1) * H_], rhs=hb, **acc)
            mm(ps[:, 56:64], lhsT=whh1[:, GN * H_:(GN + 1) * H_], rhs=hb,
               start=False, stop=True)

            ps3 = ps[:, :].rearrange("p (l c) -> p l c", l=2)  # [128,2,32]

            rz = workp.tile([128, 2, 16], F32, tag="rz", name="rz")
            nc.scalar.activation(out=rz[:, :, :], in_=ps3[:, :, 0:16],
                                 func=AF.Sigmoid)
            tt = workp.tile([128, 2, 8], F32, tag="tt", name="tt")
            nc.vector.tensor_mul(tt[:, :, :], rz[:, :, 0:8], ps3[:, :, 24:32])
            ss = workp.tile([128, 2, 8], F32, tag="ss", name="ss")
            nc.vector.tensor_add(ss[:, :, :], tt[:, :, :], ps3[:, :, 16:24])
            nn = workp.tile([128, 2, 8], F32, tag="nn", name="nn")
            nc.scalar.activation(out=nn[:, :, :], in_=ss[:, :, :], func=AF.Tanh)
            hp2 = hprev.rearrange("p (l c) -> p l c", l=2)
            dd = workp.tile([128, 2, 8], F32, tag="dd", name="dd")
            nc.vector.scalar_tensor_tensor(
                out=dd[:, :, :], in0=nn[:, :, :], scalar=-1.0, in1=hp2,
                op0=OP.mult, op1=OP.add)  # d = hprev - n
            ee = workp.tile([128, 2, 8], F32, tag="ee", name="ee")
            nc.vector.tensor_mul(ee[:, :, :], dd[:, :, :], rz[:, :, 8:16])

            if DEBUG_DUMP and tau == DEBUG_TAU:
                psc = workp.tile([128, 64], F32, tag="psc", name="psc")
                nc.vector.tensor_copy(psc[:, :], ps[:, :])
                for nm, tl in [("ps", psc), ("rz", rz), ("tt", tt),
                               ("ss", ss), ("nn", nn), ("dd", dd), ("ee", ee)]:
                    d = nc.dram_tensor(f"dbg_{nm}", list(tl.shape), F32,
                                       kind="ExternalOutput").ap()
                    flat = tl[(slice(None),) * len(tl.shape)]
                    if len(tl.shape) == 3:
                        flat = flat.rearrange("p a b -> p (a b)")
                        d = d.rearrange("p a b -> p (a b)")
                    nc.sync.dma_start(out=d, in_=flat)

            dest = entry_ap(tau).rearrange("p (l c) -> p l c", l=2)
            if tau == 0:
                nc.vector.tensor_add(dest[:, 0, :], nn[:, 0, :], ee[:, 0, :])
            elif tau == S:
                nc.vector.tensor_add(dest[:, 1, :], nn[:, 1, :], ee[:, 1, :])
            else:
                nc.vector.tensor_add(dest[:, :, :], nn[:, :, :], ee[:, :, :])

        def head_chunk(k):
            hbt = hab_tiles[k]
            xtt = xt_tiles[k]
            for j in range(C // 64):
                hps = pshp.tile([MLP_, 512], F32, tag="hps", name="hps")
                # rhs: h2 for 64 steps, t-major cols (t, b)
                nc.tensor.matmul(hps[:, :], lhsT=w1h[:, :],
                                 rhs=hbt[:, 64 * j:64 * j + 64, 8:16],
                                 start=True, stop=False)
                nc.tensor.matmul(hps[:, :], lhsT=w1x[:, :],
                                 rhs=xtt[:, :, 64 * j:64 * j + 64]
                                 .rearrange("k b t -> k t b"),
                                 start=False, stop=True)
                hdn = hdnp.tile([MLP_, 512], F32, tag="hdn", name="hdn")
                nc.scalar.activation(out=hdn[:, :], in_=hps[:, :], func=AF.Relu)
                ips = psip.tile([1, 512], F32, tag="ips", name="ips")
                nc.tensor.matmul(ips[:, :], lhsT=w2[:, :], rhs=hdn[:, :],
                                 start=True, stop=True)
                th = thp.tile([1, 512], F32, tag="th", name="th")
                nc.scalar.activation(out=th[:, :], in_=ips[:, :], func=AF.Tanh,
                                     bias=b2t[:, :])
                c0 = k * C + 64 * j
                thv = th[:, :].rearrange("p (t b) -> p b t", b=BL)
                for b in range(BL):
                    nc.sync.dma_start(
                        out=inc_sb[b:b + 1, c0:c0 + 64],
                        in_=thv[:, b, :])

        # ---------------- emission ----------------
        xt_load(0)
        xt_load(1)
        superstep(0)
        for k in range(nchunks):
            hab_tiles[k] = habp.tile([128, C, 16], F32, tag="hab", name="hab")
            xt_load(k + 2)
            for e in range(C):
                superstep(k * C + 1 + e)
            head_chunk(k)

        if DEBUG_DUMP:
            for nm, tl in [("zero16", zero16), ("prol", prol)]:
                d = nc.dram_tensor(f"dbg_{nm}", [128, 16], F32,
                                   kind="ExternalOutput").ap()
                nc.sync.dma_start(out=d, in_=tl[:, :])
            nc.sync.dma_start(out=dbg_hab_d,
                              in_=hab_tiles[0][:, :, :].rearrange(
                                  "p e c -> p (e c)"))
            nc.sync.dma_start(out=dbg_inc_d, in_=inc_sb[:, :])

        scan = bigp.tile([BL, S], F32, tag="scan", name="scan")
        nc.vector.tensor_tensor_scan(
            out=scan[:, :], data0=inc_sb[:, :], data1=inc_sb[:, :],
            initial=0.0, op0=OP.add, op1=OP.bypass)
        outb = bigp.tile([BL, S], F32, tag="outb", name="outb")
        nc.vector.tensor_scalar(
            out=outb[:, :], in0=scan[:, :], scalar1=0.125, scalar2=imf,
            op0=OP.mult, op1=OP.add)
        nc.sync.dma_start(out=y_d, in_=outb[:, :])

    nc.compile()
    return nc


def _prep_host_inputs(inputs, S):
    f = lambda k: np.asarray(inputs[k], np.float32)
    x = f("nutrition_data")[:, :S, :]  # [B, S, I]
    w_ih0, w_hh0 = f("w_ih0"), f("w_hh0")
    b_ih0, b_hh0 = f("b_ih0"), f("b_hh0")
    w_ih1, w_hh1 = f("w_ih1"), f("w_hh1")
    b_ih1, b_hh1 = f("b_ih1"), f("b_hh1")
    w1, b1 = f("w1"), f("b1")
    w2, b2 = f("w2"), f("b2")

    wih0 = np.zeros([17, 3 * H_], np.float32)
    wih0[:16] = w_ih0.T
    # aug bias row: r,z get b_ih+b_hh; n gets b_ih only (b_hn stays separate)
    wih0[16, 0:H_] = b_ih0[0:H_] + b_hh0[0:H_]
    wih0[16, H_:2 * H_] = b_ih0[H_:2 * H_] + b_hh0[H_:2 * H_]
    wih0[16, 2 * H_:3 * H_] = b_ih0[2 * H_:3 * H_]

    biasT = np.stack([
        b_hh0[2 * H_:3 * H_],                    # b_hn0 -> nh0 block
        b_ih1[0:H_] + b_hh1[0:H_],               # r1
        b_ih1[H_:2 * H_] + b_hh1[H_:2 * H_],     # z1
        b_ih1[2 * H_:3 * H_],                    # ni1
        b_hh1[2 * H_:3 * H_],                    # nh1 (b_hn1)
    ]).astype(np.float32)
    bsel = np.zeros([5, 64], np.float32)
    for r in range(5):
        bsel[r, 24 + 8 * r:32 + 8 * r] = 1.0

    w1x = np.zeros([17, MLP_], np.float32)
    w1x[:16] = w1[:, H_:].T
    w1x[16] = b1

    shared = {
        "wih0": wih0,
        "whh0": np.ascontiguousarray(w_hh0.T),
        "wih1": np.ascontiguousarray(w_ih1.T),
        "whh1": np.ascontiguousarray(w_hh1.T),
        "biasT": biasT,
        "bsel": bsel,
        "w1h": np.ascontiguousarray(w1[:, :H_].T),
        "w1x": w1x,
        "w2": np.ascontiguousarray(w2.T),
    }

    in_maps = []
    for c in range(NCORES):
        xc = x[c * BL:(c + 1) * BL]  # [8, S, 16]
        xt = np.ones([17, BL * S], np.float32)
        xt[:16] = xc.transpose(2, 0, 1).reshape(16, BL * S)  # col = b*S + t
        m = dict(shared)
        m["xt"] = xt
        in_maps.append(m)

    b2f = float(np.asarray(b2).reshape(-1)[0])
    imf = float(np.asarray(inputs["initial_metabolism"]))
    return in_maps, b2f, imf


def run(inputs, S=S_FULL, C=None, trace=False):
    from concourse import bass_utils
    if C is None:
        C = min(512, S)
    in_maps, b2f, imf = _prep_host_inputs(inputs, S)
    nc = _build_program(S, C, b2f, imf)
    res = bass_utils.run_bass_kernel_spmd(
        nc, in_maps, core_ids=list(range(NCORES)), trace=trace)
    LAST["exec_time_ns"] = res.exec_time_ns
    LAST["trace"] = res.instructions_and_trace
    LAST["results"] = res.results
    y = np.stack([res.results[c]["y"] for c in range(NCORES)])  # [8, 8, S]
    return y.reshape(B_, S, 1).astype(np.float32)


def kernel(**inputs):
    return run(inputs, S=S_FULL)

